# revision 20
# baseline (speedup 1.0000x reference)
"""OCRHead Trainium2 kernel (self-contained).

Strategy: 8-way SPMD over (batch b, image half). Each core owns 64 image rows
of one batch element. Conv3x3 is computed as 9 shifted matmuls contracting
over CIN (720, 6 k-tiles). BatchNorm statistics are all-reduced over all 8
cores; the SpatialGather context (per batch element) is all-reduced between
the 2 cores sharing a batch element. Everything else is core-local.

Layouts (per core, px = 64*128 = 8192 pixels):
  x      [MID=512 (4 part-tiles), px]  bf16     (conv out -> BN+relu in place)
  xT     [px (128-part tiles), 512]    bf16     (DVE stream-transposed, for context matmul)
  probs  [px, 19(pad 32)]              bf16     (softmax over classes, free dim)
  q      [KC=256 (2 tiles), px]        bf16
  attn   [px, 19(pad 32)] -> attnT [19(pad 32), px] bf16
"""

import os
import sys
from contextlib import ExitStack

import numpy as np

if "/opt/trn_rl_repo" not in sys.path:
    sys.path.insert(0, "/opt/trn_rl_repo")

import ml_dtypes  # noqa: E402

import concourse.bass as bass  # noqa: E402
import concourse.bacc as bacc  # noqa: E402
import concourse.tile as tile  # noqa: E402
from concourse import mybir  # noqa: E402
from concourse.bass import ds, ts  # noqa: E402
from concourse.masks import make_identity  # noqa: E402

F32 = mybir.dt.float32
BF16 = mybir.dt.bfloat16
AF = mybir.ActivationFunctionType
ALU = mybir.AluOpType
BFNP = ml_dtypes.bfloat16

# Problem constants
B = 4
CIN = 720
H = 128
W = 128
MID = 512
NCLS = 19
KC = 256
VC = 256
BN_EPS = 1e-5
NCORES = 8
NMT = MID // 128  # 4 tiles over MID


def ktiles(cin):
    out = []
    c = cin
    while c > 0:
        out.append(min(128, c))
        c -= 128
    return out


class Cfg:
    def __init__(self, cin=CIN, rows=H // 2, ncores=NCORES):
        self.cin = cin
        self.rows = rows            # image rows owned by one core (multiple of 4)
        self.ncores = ncores
        self.kts = ktiles(cin)
        self.nkt = len(self.kts)
        self.px = rows * 128        # pixels per core
        self.pxt = rows             # 128-pixel tiles per core (1 per image row)
        self.nch = rows // 4        # 512-pixel chunks per core
        self.nbn = ncores * self.px  # total elements per channel for BN stats


def build_program(cfg: Cfg):
    nc = bacc.Bacc(None, target_bir_lowering=False, debug=False,
                   num_devices=cfg.ncores, dynamic_dma_scratch_size=4096)
    NKT, KTS, PXT, NCH = cfg.nkt, cfg.kts, cfg.pxt, cfg.nch
    ROWS = cfg.rows

    # ---------------- DRAM I/O ----------------
    feats_d = nc.dram_tensor("feats_l", [cfg.cin, ROWS + 2, W + 2], BF16,
                             kind="ExternalInput")
    w3s_d = nc.dram_tensor("w3s", [NKT, NMT, 128, 9 * 128], BF16,
                           kind="ExternalInput")
    gamma_d = nc.dram_tensor("gamma_v", [MID], F32, kind="ExternalInput")
    beta_d = nc.dram_tensor("beta_v", [MID], F32, kind="ExternalInput")
    auxw_d = nc.dram_tensor("aux_wT", [MID, 32], BF16, kind="ExternalInput")
    auxb_d = nc.dram_tensor("aux_b32", [32], F32, kind="ExternalInput")
    qw_d = nc.dram_tensor("q_wT", [MID, KC], BF16, kind="ExternalInput")
    kw_d = nc.dram_tensor("k_wT", [MID, KC], BF16, kind="ExternalInput")
    kb_d = nc.dram_tensor("k_b", [KC], F32, kind="ExternalInput")
    vw_d = nc.dram_tensor("v_wT", [MID, VC], BF16, kind="ExternalInput")
    vb_d = nc.dram_tensor("v_b", [VC], F32, kind="ExternalInput")
    ow_d = nc.dram_tensor("out_wT", [VC, MID], BF16, kind="ExternalInput")
    clsw_d = nc.dram_tensor("cls_wT", [MID, 32], BF16, kind="ExternalInput")
    clsb_d = nc.dram_tensor("cls_b32", [32], F32, kind="ExternalInput")

    logits_d = nc.dram_tensor("logits_l", [NCLS, ROWS, W], F32,
                              kind="ExternalOutput")
    aux_d = nc.dram_tensor("auxl_l", [NCLS, ROWS, W], F32,
                           kind="ExternalOutput")

    # internal DRAM (collective bounce buffers + broadcast scratch)
    bn_in = nc.dram_tensor("bn_in", [128, 8], F32, kind="Internal")
    bn_out = nc.dram_tensor("bn_out", [128, 8], F32, kind="Internal",
                            addr_space="Shared")
    ctx_in = nc.dram_tensor("ctx_in", [32, 513], F32, kind="Internal")
    ctx_out = nc.dram_tensor("ctx_out", [32, 513], F32, kind="Internal")
    sc_d = nc.dram_tensor("sc_d", [1024], F32, kind="Internal")

    groups_all = [list(range(cfg.ncores))]
    groups_pair = [[2 * i, 2 * i + 1] for i in range(cfg.ncores // 2)]

    with tile.TileContext(nc) as tc:
        es = ExitStack()
        with es:
            consts = es.enter_context(tc.tile_pool(name="consts", bufs=1))

            # ---- constants / weights resident in SBUF ----
            ident = consts.tile([128, 128], F32)
            make_identity(nc, ident)
            eps_t = consts.tile([128, 1], F32)
            nc.vector.memset(eps_t, BN_EPS)

            gamma_t = consts.tile([128, NMT], F32)
            nc.sync.dma_start(out=gamma_t,
                              in_=gamma_d.ap().rearrange("(m p) -> p m", p=128))
            beta_t = consts.tile([128, NMT], F32)
            nc.sync.dma_start(out=beta_t,
                              in_=beta_d.ap().rearrange("(m p) -> p m", p=128))

            auxw_t = consts.tile([128, NMT, 32], BF16)
            nc.sync.dma_start(out=auxw_t,
                              in_=auxw_d.ap().rearrange("(m p) n -> p m n", p=128))
            clsw_t = consts.tile([128, NMT, 32], BF16)
            nc.sync.dma_start(out=clsw_t,
                              in_=clsw_d.ap().rearrange("(m p) n -> p m n", p=128))
            qw_t = consts.tile([128, NMT, KC], BF16)
            nc.sync.dma_start(out=qw_t,
                              in_=qw_d.ap().rearrange("(m p) n -> p m n", p=128))
            kw_t = consts.tile([128, NMT, KC], BF16)
            nc.sync.dma_start(out=kw_t,
                              in_=kw_d.ap().rearrange("(m p) n -> p m n", p=128))
            vw_t = consts.tile([128, NMT, VC], BF16)
            nc.sync.dma_start(out=vw_t,
                              in_=vw_d.ap().rearrange("(m p) n -> p m n", p=128))
            ow_t = consts.tile([128, VC // 128, MID], BF16)
            nc.sync.dma_start(out=ow_t,
                              in_=ow_d.ap().rearrange("(m p) n -> p m n", p=128))

            auxb_t = consts.tile([32, 1], F32)
            nc.sync.dma_start(out=auxb_t, in_=auxb_d.ap())
            clsb_t = consts.tile([32, 1], F32)
            nc.sync.dma_start(out=clsb_t, in_=clsb_d.ap())
            kb_t = consts.tile([128, 2], F32)
            nc.sync.dma_start(out=kb_t,
                              in_=kb_d.ap().rearrange("(m p) -> p m", p=128))
            vb_ap = vb_d.ap()
            vb_bc = consts.tile([32, VC], F32)
            nc.sync.dma_start(out=vb_bc,
                              in_=bass.AP(tensor=vb_ap.tensor, offset=vb_ap.offset,
                                          ap=[[0, 32]] + list(vb_ap.ap)))

            # ---- BN stat accumulators + misc small tiles ----
            sum_p = consts.tile([128, NMT, NCH], F32)
            sumsq_p = consts.tile([128, NMT, NCH], F32)
            stats_pack = consts.tile([128, 8], F32)
            stats_all = consts.tile([128, 8], F32)
            mean_t = consts.tile([128, NMT], F32)
            tmp_t = consts.tile([128, NMT], F32)
            var_t = consts.tile([128, NMT], F32)
            std_t = consts.tile([128, NMT], F32)
            rstd_t = consts.tile([128, NMT], F32)
            scale_t = consts.tile([128, NMT], F32)
            shift_t = consts.tile([128, NMT], F32)
            sc_row = consts.tile([1, 1024], F32)
            scale_bc = consts.tile([128, 1024], F32)
            esum = consts.tile([128, PXT], F32)
            esum_rb = consts.tile([128, PXT], BF16)
            esum_r = consts.tile([128, PXT], F32)
            asum = consts.tile([128, PXT], F32)
            asum_r = consts.tile([128, PXT], F32)

            # ---- persistent activation tensors ----
            xpool = es.enter_context(tc.tile_pool(name="xp", bufs=1))
            xs = [xpool.tile([128, cfg.px], BF16, name=f"x{mt}")
                  for mt in range(NMT)]

            xT_es = ExitStack()
            xTp = xT_es.enter_context(
                tc.tile_pool(name="xTp", bufs=1, side="right"))
            xT = xTp.tile([128, PXT, MID], BF16, name="xT")

            # =========================================================
            # Phase 1: conv3x3 (+ BN partial stats, + stream-transpose)
            # =========================================================
            with tc.tile_pool(name="w3p", bufs=1) as w3p, \
                 tc.tile_pool(name="fp", bufs=2) as fp, \
                 tc.tile_pool(name="sqp", bufs=2) as sqp, \
                 tc.tile_pool(name="cvps", bufs=4, space="PSUM") as cvps:
                n_acc = 9 * NKT
                for mt in range(NMT):
                    w3t = [w3p.tile([128, 9 * 128], BF16, tag=f"w3_{kt}",
                                    name=f"w3_{kt}") for kt in range(NKT)]
                    for kt in range(NKT):
                        nc.sync.dma_start(out=w3t[kt][0:KTS[kt], :],
                                          in_=w3s_d.ap()[kt, mt, 0:KTS[kt], :])
                    for rg in range(NCH):
                        fts = [fp.tile([128, 6, W + 2], BF16, tag=f"ft{kt}",
                                       name=f"ft{kt}") for kt in range(NKT)]
                        for kt in range(NKT):
                            nc.sync.dma_start(
                                out=fts[kt][0:KTS[kt]],
                                in_=feats_d.ap()[kt * 128:kt * 128 + KTS[kt],
                                                 rg * 4:rg * 4 + 6, :])
                        P = cvps.tile([128, 512], F32, tag="cv", name="Pcv")
                        i = 0
                        for dy in range(3):
                            for dx in range(3):
                                for kt in range(NKT):
                                    nc.tensor.matmul(
                                        P,
                                        w3t[kt][0:KTS[kt],
                                                ds((dy * 3 + dx) * 128, 128)],
                                        fts[kt][0:KTS[kt], dy:dy + 4,
                                                dx:dx + 128],
                                        start=(i == 0), stop=(i == n_acc - 1))
                                    i += 1
                        # epilogue: copy to x (bf16) + per-channel sum/sumsq
                        nc.scalar.activation(out=xs[mt][:, ds(rg * 512, 512)],
                                             in_=P, func=AF.Copy,
                                             accum_out=sum_p[:, mt, rg:rg + 1])
                        sq = sqp.tile([128, 512], BF16, tag="sq", name="sq")
                        nc.scalar.activation(out=sq, in_=P, func=AF.Square,
                                             accum_out=sumsq_p[:, mt, rg:rg + 1])
                    # stream-transpose this mt's raw x into xT (pre-BN values;
                    # BN applied to xT separately later). StreamTranspose
                    # flips each 32x32 block in place; the tensor_copies then
                    # permute blocks to complete the full transpose.
                    btx = w3p.tile([128, cfg.px], BF16, tag="btx", name="btx",
                                   bufs=1)
                    nc.vector.transpose(out=btx, in_=xs[mt])
                    xvb = btx.rearrange("p (t s w) -> p t s w",
                                        t=PXT, s=4, w=32)
                    for ci in range(4):
                        for s in range(4):
                            nc.vector.tensor_copy(
                                out=xT[32 * s:32 * s + 32, :,
                                       ds(mt * 128 + ci * 32, 32)],
                                in_=xvb[32 * ci:32 * ci + 32, :, s, :])

            # =========================================================
            # Phase 2: BN stats all-reduce + scale/shift
            # =========================================================
            nc.vector.tensor_reduce(out=stats_pack[:, 0:NMT], in_=sum_p,
                                    axis=mybir.AxisListType.X, op=ALU.add)
            nc.vector.tensor_reduce(out=stats_pack[:, NMT:2 * NMT], in_=sumsq_p,
                                    axis=mybir.AxisListType.X, op=ALU.add)
            nc.sync.dma_start(out=bn_in.ap(), in_=stats_pack)
            nc.gpsimd.collective_compute("AllReduce", ALU.add,
                                         replica_groups=groups_all,
                                         ins=[bn_in.ap()], outs=[bn_out.ap()])
            nc.sync.dma_start(out=stats_all, in_=bn_out.ap())

            inv_n = 1.0 / float(cfg.nbn)
            nc.vector.tensor_scalar_mul(mean_t, stats_all[:, 0:NMT], inv_n)
            nc.vector.tensor_mul(tmp_t, mean_t, mean_t)
            nc.vector.scalar_tensor_tensor(out=var_t,
                                           in0=stats_all[:, NMT:2 * NMT],
                                           scalar=inv_n, in1=tmp_t,
                                           op0=ALU.mult, op1=ALU.subtract)
            nc.scalar.activation(out=std_t, in_=var_t, func=AF.Sqrt, bias=eps_t)
            nc.vector.reciprocal(rstd_t, std_t)
            nc.vector.tensor_mul(scale_t, rstd_t, gamma_t)
            nc.vector.tensor_mul(tmp_t, mean_t, scale_t)
            nc.vector.tensor_sub(shift_t, beta_t, tmp_t)

            # scale/shift -> row layout -> broadcast tile [128, 1024]
            with tc.tile_pool(name="scps", bufs=1, space="PSUM") as scps:
                pscale = scps.tile([1, 512], F32, tag="pscale", name="pscale")
                pshift = scps.tile([1, 512], F32, tag="pshift", name="pshift")
                for mt in range(NMT):
                    nc.tensor.transpose(out=pscale[:, ds(mt * 128, 128)],
                                        in_=scale_t[:, mt:mt + 1],
                                        identity=ident)
                    nc.tensor.transpose(out=pshift[:, ds(mt * 128, 128)],
                                        in_=shift_t[:, mt:mt + 1],
                                        identity=ident)
                nc.vector.tensor_copy(sc_row[:, 0:512], pscale)
                nc.vector.tensor_copy(sc_row[:, 512:1024], pshift)
            nc.sync.dma_start(out=sc_d.ap(), in_=sc_row)
            sc_ap = sc_d.ap()
            nc.sync.dma_start(out=scale_bc,
                              in_=bass.AP(tensor=sc_ap.tensor, offset=sc_ap.offset,
                                          ap=[[0, 128]] + list(sc_ap.ap)))

            # =========================================================
            # Phase 3: normalize x in place (ACT) + xT scale/shift (DVE)
            # =========================================================
            for mt in range(NMT):
                for ch in range(NCH):
                    sl = xs[mt][:, ds(ch * 512, 512)]
                    nc.scalar.activation(out=sl, in_=sl, func=AF.Relu,
                                         bias=shift_t[:, mt:mt + 1],
                                         scale=scale_t[:, mt:mt + 1])
            for t in range(PXT):
                v = xT[:, t, :]
                nc.vector.tensor_mul(v, v, scale_bc[:, 0:512])
                nc.vector.tensor_add(v, v, scale_bc[:, 512:1024])

            # =========================================================
            # Phase 4: aux head + probs (softmax over classes), q proj
            # =========================================================
            aux_v = aux_d.ap().rearrange("k h w -> k (h w)")
            logits_v = logits_d.ap().rearrange("k h w -> k (h w)")
            qp = es.enter_context(tc.tile_pool(name="qp", bufs=1))
            q_sb = [qp.tile([128, cfg.px], BF16, name=f"q{i}") for i in range(2)]
            stp = es.enter_context(tc.tile_pool(name="stp", bufs=2))
            pp_es = ExitStack()
            probs_pool = pp_es.enter_context(tc.tile_pool(name="pp", bufs=1))
            probs = probs_pool.tile([128, PXT, 32], BF16, name="probs")
            probsT = probs_pool.tile([128, PXT, 32], BF16, name="probsT")
            nc.vector.memset(probs, 0.0)
            aux_es = ExitStack()
            auxps = aux_es.enter_context(
                tc.tile_pool(name="auxps", bufs=2, space="PSUM"))
            for ch in range(NCH):
                Pa = auxps.tile([32, 512], F32, tag="Pa", name="Pa")
                for kt in range(NMT):
                    nc.tensor.matmul(Pa, auxw_t[:, kt, :],
                                     xs[kt][:, ds(ch * 512, 512)],
                                     start=(kt == 0), stop=(kt == NMT - 1))
                ast = stp.tile([32, 512], F32, tag="ast", name="ast")
                nc.vector.tensor_scalar_add(out=ast, in0=Pa, scalar1=auxb_t)
                nc.sync.dma_start(out=aux_v[:, ds(ch * 512, 512)],
                                  in_=ast[0:19, :])
                bta = stp.tile([32, 512], F32, tag="bta", name="bta")
                nc.vector.transpose(out=bta, in_=ast)
                av = bta.rearrange("p (t s w) -> p t s w", t=4, s=4, w=32)
                for s in range(4):
                    nc.vector.tensor_copy(
                        out=probsT[32 * s:32 * s + 32, ch * 4:(ch + 1) * 4, :],
                        in_=av[:, :, s, :])
            for t in range(PXT):
                nc.scalar.activation(out=probs[:, t, 0:19],
                                     in_=probsT[:, t, 0:19], func=AF.Exp,
                                     accum_out=esum[:, t:t + 1])
            nc.vector.reciprocal(esum_r, esum)
            nc.vector.tensor_copy(esum_rb, esum_r)
            aux_es.close()

            # q projection: q[kc, px]
            q_es2 = ExitStack()
            qps = q_es2.enter_context(
                tc.tile_pool(name="qps", bufs=3, space="PSUM"))
            for kct in range(2):
                for ch in range(NCH):
                    Pq = qps.tile([128, 512], F32, tag="Pq", name="Pq")
                    for kt in range(NMT):
                        nc.tensor.matmul(Pq, qw_t[:, kt, ds(kct * 128, 128)],
                                         xs[kt][:, ds(ch * 512, 512)],
                                         start=(kt == 0), stop=(kt == NMT - 1))
                    nc.vector.tensor_copy(q_sb[kct][:, ds(ch * 512, 512)], Pq)
            q_es2.close()

            # =========================================================
            # Phase 5: spatial-gather context + pair all-reduce
            # =========================================================
            # xT <- relu(xT)*esum_r  (fold softmax-denominator into x rows)
            for t in range(PXT):
                nc.vector.tensor_scalar(out=xT[:, t, :], in0=xT[:, t, :],
                                        scalar1=0.0,
                                        scalar2=esum_r[:, t:t + 1],
                                        op0=ALU.max, op1=ALU.mult)
            ctx_es = ExitStack()
            ctxps = ctx_es.enter_context(
                tc.tile_pool(name="ctxps", bufs=1, space="PSUM"))
            Pc = ctxps.tile([32, 512], F32, tag="Pc", name="Pc")
            Pd = ctxps.tile([32, 1], F32, tag="Pd", name="Pd")
            for t in range(PXT):
                nc.tensor.matmul(Pc, probs[:, t, :], xT[:, t, :],
                                 start=(t == 0), stop=(t == PXT - 1))
            for t in range(PXT):
                nc.tensor.matmul(Pd, probs[:, t, :], esum_rb[:, t:t + 1],
                                 start=(t == 0), stop=(t == PXT - 1))
            ctx_sb = consts.tile([32, 513], F32)
            nc.vector.tensor_copy(ctx_sb[:, 0:512], Pc)
            nc.vector.tensor_copy(ctx_sb[:, 512:513], Pd)
            ctx_es.close()
            nc.sync.dma_start(out=ctx_in.ap(), in_=ctx_sb)
            nc.gpsimd.collective_compute("AllReduce", ALU.add,
                                         replica_groups=groups_pair,
                                         ins=[ctx_in.ap()], outs=[ctx_out.ap()])
            ctx_all = consts.tile([32, 513], F32)
            nc.sync.dma_start(out=ctx_all, in_=ctx_out.ap())

            dmax = consts.tile([32, 1], F32)
            dr = consts.tile([32, 1], F32)
            ctx_n = consts.tile([32, 512], F32)
            ctx_nb = consts.tile([32, 512], BF16)
            nc.vector.tensor_scalar_max(dmax, ctx_all[:, 512:513], 1e-6)
            nc.vector.reciprocal(dr, dmax)
            nc.vector.tensor_scalar_mul(ctx_n, ctx_all[:, 0:512], dr)
            nc.vector.tensor_copy(ctx_nb, ctx_n)
            ctxT = consts.tile([128, NMT, 32], BF16)
            btc = consts.tile([32, 512], BF16)
            nc.vector.transpose(out=btc, in_=ctx_nb)
            cv = btc.rearrange("p (t s w) -> p t s w", t=NMT, s=4, w=32)
            for s in range(4):
                nc.vector.tensor_copy(out=ctxT[32 * s:32 * s + 32, :, :],
                                      in_=cv[:, :, s, :])
            pp_es.close()  # probs dead
            xT_es.close()  # xT dead (right-side stack)

            # =========================================================
            # Phase 6: k/v projections of context
            # =========================================================
            k_sb = [consts.tile([128, 19], BF16, name=f"k{i}") for i in range(2)]
            v_sb = consts.tile([32, VC], BF16)
            kv_es = ExitStack()
            kvps = kv_es.enter_context(
                tc.tile_pool(name="kvps", bufs=1, space="PSUM"))
            for mt in range(2):
                Pk = kvps.tile([128, 19], F32, tag=f"Pk{mt}", name=f"Pk{mt}")
                for kt in range(NMT):
                    nc.tensor.matmul(Pk, kw_t[:, kt, ds(mt * 128, 128)],
                                     ctxT[:, kt, 0:19],
                                     start=(kt == 0), stop=(kt == NMT - 1))
                nc.vector.tensor_scalar(out=k_sb[mt], in0=Pk,
                                        scalar1=kb_t[:, mt:mt + 1],
                                        scalar2=1.0 / 16.0,
                                        op0=ALU.add, op1=ALU.mult)
            Pv = kvps.tile([32, VC], F32, tag="Pv", name="Pv")
            for kt in range(NMT):
                nc.tensor.matmul(Pv, ctxT[:, kt, :], vw_t[:, kt, :],
                                 start=(kt == 0), stop=(kt == NMT - 1))
            nc.vector.tensor_add(v_sb, Pv, vb_bc)
            kv_es.close()

            # =========================================================
            # Phase 7: attention + aggregation + out-proj + cls head
            # =========================================================
            at_es = ExitStack()
            atp = at_es.enter_context(tc.tile_pool(name="atp", bufs=1))
            attn = atp.tile([128, PXT, 32], BF16, name="attn")
            attnT = atp.tile([32, cfg.px], BF16, name="attnT")
            nc.vector.memset(attn, 0.0)
            attnps = at_es.enter_context(
                tc.tile_pool(name="attnps", bufs=2, space="PSUM"))
            for t in range(PXT):
                Pat = attnps.tile([128, 19], F32, tag="Pat", name="Pat")
                for kct in range(2):
                    nc.tensor.matmul(Pat, q_sb[kct][:, ds(t * 128, 128)],
                                     k_sb[kct],
                                     start=(kct == 0), stop=(kct == 1))
                nc.scalar.activation(out=attn[:, t, 0:19], in_=Pat,
                                     func=AF.Exp, accum_out=asum[:, t:t + 1])
            nc.vector.reciprocal(asum_r, asum)
            for t in range(PXT):
                nc.vector.tensor_scalar_mul(attn[:, t, 0:19],
                                            attn[:, t, 0:19],
                                            asum_r[:, t:t + 1])
            btt = atp.tile([128, PXT * 32], BF16, name="btt")
            nc.vector.transpose(out=btt,
                                in_=attn.rearrange("p t w -> p (t w)"))
            btv = btt.rearrange("p (t w) -> p t w", t=PXT, w=32)
            atv = attnT.rearrange("p (t s w) -> p t s w", t=PXT, s=4, w=32)
            for s in range(4):
                nc.vector.tensor_copy(out=atv[:, :, s, :],
                                      in_=btv[32 * s:32 * s + 32, :, :])

            aggps = at_es.enter_context(
                tc.tile_pool(name="aggps", bufs=2, space="PSUM"))
            aggsb = at_es.enter_context(tc.tile_pool(name="aggsb", bufs=2))
            objps = at_es.enter_context(
                tc.tile_pool(name="objps", bufs=2, space="PSUM"))
            clsps = at_es.enter_context(
                tc.tile_pool(name="clsps", bufs=2, space="PSUM"))
            for ch in range(NCH):
                aggs = []
                for mtv in range(2):
                    Pg = aggps.tile([128, 512], F32, tag="Pg", name="Pg")
                    nc.tensor.matmul(Pg, v_sb[0:19, ds(mtv * 128, 128)],
                                     attnT[0:19, ds(ch * 512, 512)],
                                     start=True, stop=True)
                    ag = aggsb.tile([128, 512], BF16, tag=f"ag{mtv}",
                                    name=f"ag{mtv}")
                    nc.vector.tensor_copy(ag, Pg)
                    aggs.append(ag)
                for mto in range(NMT):
                    Po = objps.tile([128, 512], F32, tag="Po", name="Po")
                    for ktv in range(2):
                        nc.tensor.matmul(Po, ow_t[:, ktv, ds(mto * 128, 128)],
                                         aggs[ktv],
                                         start=(ktv == 0), stop=(ktv == 1))
                    sl = xs[mto][:, ds(ch * 512, 512)]
                    nc.vector.tensor_add(sl, Po, sl)
                Pl = clsps.tile([32, 512], F32, tag="Pl", name="Pl")
                for kt in range(NMT):
                    nc.tensor.matmul(Pl, clsw_t[:, kt, :],
                                     xs[kt][:, ds(ch * 512, 512)],
                                     start=(kt == 0), stop=(kt == NMT - 1))
                lst = stp.tile([32, 512], F32, tag="lst", name="lst")
                nc.vector.tensor_scalar_add(out=lst, in0=Pl, scalar1=clsb_t)
                nc.sync.dma_start(out=logits_v[:, ds(ch * 512, 512)],
                                  in_=lst[0:19, :])
            at_es.close()

    nc.compile()
    return nc


# =====================================================================
# Host side: shard/pack inputs, run SPMD, gather outputs
# =====================================================================

def prep_shared(inputs, cfg: Cfg):
    """Core-independent packed weights."""
    w3 = np.asarray(inputs["w3"], np.float32)          # [MID, CIN, 3, 3]
    NKT, KTS = cfg.nkt, cfg.kts
    w3_t = np.ascontiguousarray(w3.transpose(1, 2, 3, 0)).reshape(cfg.cin, 9, MID)
    w3s = np.zeros([NKT, NMT, 128, 9 * 128], BFNP)
    for kt in range(NKT):
        blk = w3_t[kt * 128: kt * 128 + KTS[kt]]       # [KP, 9, MID]
        for mt in range(NMT):
            w3s[kt, mt, 0:KTS[kt]] = blk[:, :, mt * 128:(mt + 1) * 128] \
                .reshape(KTS[kt], 9 * 128).astype(BFNP)

    def padT(wm):  # [19, MID] -> [MID, 32] bf16, zero padded
        out = np.zeros([MID, 32], BFNP)
        out[:, :19] = np.asarray(wm, np.float32).T.astype(BFNP)
        return out

    def pad32(v):
        out = np.zeros([32], np.float32)
        out[:19] = np.asarray(v, np.float32)
        return out

    shared = {
        "w3s": w3s,
        "gamma_v": np.asarray(inputs["gamma"], np.float32),
        "beta_v": np.asarray(inputs["beta"], np.float32),
        "aux_wT": padT(inputs["aux_w"]),
        "aux_b32": pad32(inputs["aux_b"]),
        "q_wT": np.asarray(inputs["q_w"], np.float32).T.astype(BFNP).copy(),
        "k_wT": np.asarray(inputs["k_w"], np.float32).T.astype(BFNP).copy(),
        "k_b": np.asarray(inputs["k_b"], np.float32),
        "v_wT": np.asarray(inputs["v_w"], np.float32).T.astype(BFNP).copy(),
        "v_b": np.asarray(inputs["v_b"], np.float32),
        "out_wT": np.asarray(inputs["out_w"], np.float32).T.astype(BFNP).copy(),
        "cls_wT": padT(inputs["cls_w"]),
        "cls_b32": pad32(inputs["cls_b"]),
    }
    return shared


def prep_core_feats(feats, b, half, cfg: Cfg):
    """Zero-padded (halo rows + W borders) bf16 shard for one core."""
    rows = cfg.rows
    r0 = half * rows
    fl = np.zeros([cfg.cin, rows + 2, W + 2], BFNP)
    lo = max(0, r0 - 1)
    hi = min(feats.shape[2], r0 + rows + 1)
    fl[:, lo - (r0 - 1):hi - (r0 - 1), 1:W + 1] = \
        feats[b, :, lo:hi, :].astype(BFNP)
    return fl


def make_in_maps(inputs, cfg: Cfg):
    shared = prep_shared(inputs, cfg)
    feats = np.asarray(inputs["feats"], np.float32)
    in_maps = []
    for core in range(cfg.ncores):
        b, half = core // 2, core % 2
        m = dict(shared)
        m["feats_l"] = prep_core_feats(feats, b, half, cfg)
        in_maps.append(m)
    return in_maps


_CACHED = {}


def _get_program(cfg: Cfg):
    key = (cfg.cin, cfg.rows, cfg.ncores)
    if key not in _CACHED:
        _CACHED[key] = build_program(cfg)
    return _CACHED[key]


def gather_outputs(results, cfg: Cfg):
    logits = np.zeros([B, NCLS, H, W], np.float32)
    aux = np.zeros([B, NCLS, H, W], np.float32)
    for core in range(cfg.ncores):
        b, half = core // 2, core % 2
        r0 = half * cfg.rows
        logits[b, :, r0:r0 + cfg.rows, :] = results[core]["logits_l"]
        aux[b, :, r0:r0 + cfg.rows, :] = results[core]["auxl_l"]
    return logits, aux


def kernel(**inputs):
    from concourse import bass_utils
    cfg = Cfg()
    nc = _get_program(cfg)
    in_maps = make_in_maps(inputs, cfg)
    res = bass_utils.run_bass_kernel_spmd(nc, in_maps,
                                          core_ids=list(range(cfg.ncores)))
    return gather_outputs(res.results, cfg)
